# revision 7
# baseline (speedup 1.0000x reference)
"""Trainium2 Bass kernel for the E74 checkpointed delta-rule cell.

Math (per batch b):
    k,v,q = x @ W_{k,v,q}^T            # [T, N] each
    kn = k / (||k|| + 1e-6)
    scan over t: r = S kn_t ; S += outer(v_t - r, kn_t) ; z = S q_t
                 out_t = z * silu(z)

Chunked parallel form (chunk C=128, per batch, chunk index c):
    A  = stril(KN KN^T)   (strict lower, within chunk)
    T  = (I + A)^{-1}     (unit lower triangular inverse, Newton doubling)
    U  = T (V - KN S0^T)
    O  = Q S0^T + (tril_incl(Q KN^T)) U
    S1^T = S0^T + KN^T U
    out = O * silu(O)

Sharding: data parallel over batch B=32 -> 8 cores x 4 batches.
"""

from contextlib import ExitStack

import numpy as np

import concourse.bacc as bacc
import concourse.mybir as mybir
import concourse.tile as tile
from concourse import masks
from concourse.bass_utils import run_bass_kernel_spmd

F32 = mybir.dt.float32
F32R = mybir.dt.float32r
BF16 = mybir.dt.bfloat16
AT = mybir.ActivationFunctionType
OP = mybir.AluOpType

T, B, DIM, N = 512, 32, 1024, 256
NCORES = 8
BL = B // NCORES  # local batches per core = 4
C = 128  # chunk length
NCH = T // C  # chunks per batch = 4
NT = N // 128  # n tiles = 2
DT = DIM // 128  # d tiles = 8
P3N = 3 * N  # packed projection width (k|v|q)

# knobs
N_BF16_ITERS = 2  # Newton doubling iterations in bf16 (order 2^(iters+1)-1)
REFINE_FP32 = True  # final Newton step in fp32
REPS = 1  # replicate whole computation on-device (timing experiments)


def build_body(nc, tc, ctx, xT, wT, s0T, out):
    consts = ctx.enter_context(tc.tile_pool(name="consts", bufs=1))
    wpool = ctx.enter_context(tc.tile_pool(name="w", bufs=1))
    xpool = ctx.enter_context(tc.tile_pool(name="x", bufs=12))
    kvq = ctx.enter_context(tc.tile_pool(name="kvq", bufs=2))
    state = ctx.enter_context(tc.tile_pool(name="state", bufs=2))
    pre = ctx.enter_context(tc.tile_pool(name="pre", bufs=10))
    dbl = ctx.enter_context(tc.tile_pool(name="dbl", bufs=3))
    seq = ctx.enter_context(tc.tile_pool(name="seq", bufs=4))
    nrm = ctx.enter_context(tc.tile_pool(name="nrm", bufs=2))
    proj_ps = ctx.enter_context(tc.tile_pool(name="proj_ps", bufs=2, space="PSUM"))
    work_ps = ctx.enter_context(tc.tile_pool(name="work_ps", bufs=6, space="PSUM"))

    # constants
    ident = consts.tile([128, 128], F32, tag="ident")
    masks.make_identity(nc, ident[:])
    ident2 = consts.tile([128, 128], F32, tag="ident2")
    masks.make_identity(nc, ident2[:])
    nc.vector.tensor_scalar_mul(ident2[:], ident2[:], 2.0)
    mask_negL = consts.tile([128, 128], F32, tag="mnl")  # -1 on strict lower
    masks.make_lower_triangular(nc, mask_negL[:], val=-1.0, diag=False)
    mask_negU = consts.tile([128, 128], F32, tag="mnu")  # -1 on strict upper
    masks.make_upper_triangular(nc, mask_negU[:], val=-1.0, diag=False)
    mask_U_excl = consts.tile([128, 128], F32, tag="mue")  # 1 on strict upper
    masks.make_upper_triangular(nc, mask_U_excl[:], val=1.0, diag=False)
    mask_U_incl = consts.tile([128, 128], F32, tag="mui")  # 1 on upper+diag
    masks.make_upper_triangular(nc, mask_U_incl[:], val=1.0, diag=True)
    ident_r = consts.tile([128, 128], F32R, tag="identr")
    nc.vector.tensor_copy(ident_r[:], ident[:])
    ones_col_f = consts.tile([128, 1], F32, tag="onescf")
    nc.vector.memset(ones_col_f[:], 1.0)
    ones_col = consts.tile([128, 1], F32R, tag="onesc")
    nc.vector.tensor_copy(ones_col[:], ones_col_f[:])
    ones_row_f = consts.tile([1, 128], F32, tag="onesrf")
    nc.vector.memset(ones_row_f[:], 1.0)
    ones_row = consts.tile([1, 128], F32R, tag="onesr")
    nc.vector.tensor_copy(ones_row[:], ones_row_f[:])

    # weights resident: 8 tiles [128, 768]
    w_sb = []
    for d in range(DT):
        wt = wpool.tile([128, P3N], F32R, tag=f"w{d}")
        nc.sync.dma_start(wt[:], wT[d * 128 : (d + 1) * 128, :])
        w_sb.append(wt)

    for rep in range(REPS):
        for b in range(BL):
            # ---- state init ----
            ST = []
            for jt in range(NT):
                st = state.tile([128, N], F32R, tag=f"st{jt}")
                nc.sync.dma_start(st[:], s0T[b, jt * 128 : (jt + 1) * 128, :])
                ST.append(st)

            # ---- x tiles for this batch: [d-tile 128, T] ----
            xt = []
            for d in range(DT):
                x_sb = xpool.tile([128, T], F32R, tag="x")
                nc.sync.dma_start(x_sb[:], xT[d * 128 : (d + 1) * 128, b, :])
                xt.append(x_sb)

            # ---- projections: out[n, t] = sum_d wT[d, n] * xT[d, t] ----
            KT, VT, QT = [None] * NT, [None] * NT, [None] * NT
            for p in range(3):  # k, v, q
                for nt in range(NT):
                    ps = proj_ps.tile([128, T], F32, tag="proj")
                    col0 = p * N + nt * 128
                    for d in range(DT):
                        nc.tensor.matmul(
                            ps[:],
                            w_sb[d][:, col0 : col0 + 128],
                            xt[d][:],
                            start=(d == 0),
                            stop=(d == DT - 1),
                        )
                    dst = kvq.tile([128, T], F32R, tag=f"kvq{p}{nt}")
                    nc.any.tensor_copy(dst[:], ps[:])
                    [KT, VT, QT][p][nt] = dst

            # ---- normalize k rows: kn = k / (||k|| + 1e-6) over n ----
            ssq = work_ps.tile([1, T], F32, tag="ps")
            for nt in range(NT):
                sq = nrm.tile([128, T], F32R, tag="sq")
                nc.vector.tensor_mul(sq[:], KT[nt][:], KT[nt][:])
                nc.tensor.matmul(
                    ssq[:],
                    ones_col[:],
                    sq[:],
                    start=(nt == 0),
                    stop=(nt == NT - 1),
                )
            nv = nrm.tile([1, T], F32, tag="nv")
            nc.scalar.sqrt(nv[:], ssq[:])
            nc.vector.tensor_scalar_add(nv[:], nv[:], 1e-6)
            inv_f = nrm.tile([1, T], F32, tag="invf")
            nc.vector.reciprocal(inv_f[:], nv[:])
            inv = nrm.tile([1, T], F32R, tag="inv")
            nc.vector.tensor_copy(inv[:], inv_f[:])
            bc = work_ps.tile([128, T], F32, tag="ps")
            nc.tensor.matmul(bc[:], ones_row[:], inv[:], start=True, stop=True)
            for nt in range(NT):
                nc.vector.tensor_mul(KT[nt][:], KT[nt][:], bc[:])

            # ---- per chunk precompute ----
            TTm, GTm, Knat, Vnat = [], [], [], []
            for c in range(NCH):
                cs = slice(c * C, (c + 1) * C)
                # pair window for wide (free=256) self-products
                pw0 = c if c < NCH - 1 else c - 1
                off = 0 if c < NCH - 1 else 128
                pws = slice(pw0 * C, (pw0 + 2) * C)

                # K, V natural via PE transpose
                kn_t = pre.tile([128, N], F32R, tag="knat")
                vn_t = pre.tile([128, N], F32R, tag="vnat")
                for nt in range(NT):
                    tp = work_ps.tile([128, 128], F32R, tag="ps")
                    nc.tensor.transpose(tp[:], KT[nt][:, cs], ident_r[:])
                    nc.any.tensor_copy(kn_t[:, nt * 128 : (nt + 1) * 128], tp[:])
                    tp2 = work_ps.tile([128, 128], F32R, tag="ps")
                    nc.tensor.transpose(tp2[:], VT[nt][:, cs], ident_r[:])
                    nc.any.tensor_copy(vn_t[:, nt * 128 : (nt + 1) * 128], tp2[:])
                Knat.append(kn_t)
                Vnat.append(vn_t)

                # KK chunk-diagonal block (wide): kk[c1, c2] = kn_c1 . kn_c2
                kk = work_ps.tile([128, 2 * C], F32, tag="ps")
                for nt in range(NT):
                    nc.tensor.matmul(
                        kk[:],
                        KT[nt][:, cs],
                        KT[nt][:, pws],
                        start=(nt == 0),
                        stop=(nt == NT - 1),
                    )
                kkb = kk[:, off : off + 128]

                # L_up = I + striu(KK);  X0 = I - stril(KK);  X0T = I - striu(KK)
                lup = dbl.tile([128, 128], F32, tag="lup")
                nc.vector.tensor_mul(lup[:], kkb, mask_U_excl[:])
                nc.vector.tensor_add(lup[:], lup[:], ident[:])
                x0 = dbl.tile([128, 128], F32, tag="x0")
                nc.vector.tensor_mul(x0[:], kkb, mask_negL[:])
                nc.vector.tensor_add(x0[:], x0[:], ident[:])
                x0t = dbl.tile([128, 128], F32, tag="x0t")
                nc.vector.tensor_mul(x0t[:], kkb, mask_negU[:])
                nc.vector.tensor_add(x0t[:], x0t[:], ident[:])

                lup_bf = dbl.tile([128, 128], BF16, tag="lupb")
                nc.any.tensor_copy(lup_bf[:], lup[:])
                xb = dbl.tile([128, 128], BF16, tag="xb")
                nc.any.tensor_copy(xb[:], x0[:])
                xtb = dbl.tile([128, 128], BF16, tag="xtb")
                nc.any.tensor_copy(xtb[:], x0t[:])

                # Newton doubling in bf16: X <- X(2I - M X), tracking X and X^T
                for _ in range(N_BF16_ITERS):
                    mx = work_ps.tile([128, 128], F32, tag="ps")
                    nc.tensor.matmul(mx[:], lup_bf[:], xb[:], start=True, stop=True)
                    t2 = dbl.tile([128, 128], BF16, tag="t2")
                    nc.vector.scalar_tensor_tensor(
                        t2[:], mx[:], -1.0, ident2[:], op0=OP.mult, op1=OP.add
                    )
                    xps = work_ps.tile([128, 128], F32, tag="ps")
                    nc.tensor.matmul(xps[:], xtb[:], t2[:], start=True, stop=True)
                    xtps = work_ps.tile([128, 128], F32, tag="ps")
                    nc.tensor.matmul(xtps[:], t2[:], xtb[:], start=True, stop=True)
                    xb = dbl.tile([128, 128], BF16, tag="xb")
                    nc.any.tensor_copy(xb[:], xps[:])
                    xtb = dbl.tile([128, 128], BF16, tag="xtb")
                    nc.any.tensor_copy(xtb[:], xtps[:])

                tt = pre.tile([128, 128], F32R, tag="tt")
                if REFINE_FP32:
                    # one fp32 Newton step; only T^T = (2I - M X)^T X^T needed
                    x32 = dbl.tile([128, 128], F32, tag="x32")
                    nc.any.tensor_copy(x32[:], xb[:])
                    xt32 = dbl.tile([128, 128], F32, tag="xt32")
                    nc.any.tensor_copy(xt32[:], xtb[:])
                    mx = work_ps.tile([128, 128], F32, tag="ps")
                    nc.tensor.matmul(mx[:], lup[:], x32[:], start=True, stop=True)
                    t2f = dbl.tile([128, 128], F32, tag="t2f")
                    nc.vector.scalar_tensor_tensor(
                        t2f[:], mx[:], -1.0, ident2[:], op0=OP.mult, op1=OP.add
                    )
                    ttp = work_ps.tile([128, 128], F32, tag="ps")
                    nc.tensor.matmul(ttp[:], t2f[:], xt32[:], start=True, stop=True)
                    nc.any.tensor_copy(tt[:], ttp[:])
                else:
                    nc.any.tensor_copy(tt[:], xtb[:])
                TTm.append(tt)

                # H' = KN Q^T (wide);  G^T = triu_incl(H')
                hp = work_ps.tile([128, 2 * C], F32, tag="ps")
                for nt in range(NT):
                    nc.tensor.matmul(
                        hp[:],
                        KT[nt][:, cs],
                        QT[nt][:, pws],
                        start=(nt == 0),
                        stop=(nt == NT - 1),
                    )
                gt = pre.tile([128, 128], F32R, tag="gt")
                nc.vector.tensor_mul(gt[:], hp[:, off : off + 128], mask_U_incl[:])
                GTm.append(gt)

            # ---- sequential scan over chunks of this batch ----
            for c in range(NCH):
                cs = slice(c * C, (c + 1) * C)
                # W1 = V - KN S0^T
                kns = work_ps.tile([128, N], F32, tag="ps")
                for jt in range(NT):
                    nc.tensor.matmul(
                        kns[:],
                        KT[jt][:, cs],
                        ST[jt][:],
                        start=(jt == 0),
                        stop=(jt == NT - 1),
                    )
                w1 = seq.tile([128, N], F32R, tag="w1")
                nc.vector.scalar_tensor_tensor(
                    w1[:], kns[:], -1.0, Vnat[c][:], op0=OP.mult, op1=OP.add
                )
                # U = T W1
                ups = work_ps.tile([128, N], F32, tag="ps")
                nc.tensor.matmul(
                    ups[:], TTm[c][:], w1[:], start=True, stop=True
                )
                u = seq.tile([128, N], F32R, tag="u")
                nc.any.tensor_copy(u[:], ups[:])
                # O = Q S0^T + G U
                ops = work_ps.tile([128, N], F32, tag="ps")
                for jt in range(NT):
                    nc.tensor.matmul(
                        ops[:],
                        QT[jt][:, cs],
                        ST[jt][:],
                        start=(jt == 0),
                        stop=False,
                    )
                nc.tensor.matmul(
                    ops[:], GTm[c][:], u[:], start=False, stop=True
                )
                # out = O * silu(O)
                sl = seq.tile([128, N], F32, tag="sl")
                nc.scalar.activation(sl[:], ops[:], AT.Silu)
                og = seq.tile([128, N], F32, tag="og")
                nc.vector.tensor_mul(og[:], sl[:], ops[:])
                if rep == REPS - 1:
                    nc.sync.dma_start(out[b, cs, :], og[:])
                # S^T += KN^T U
                for jt in range(NT):
                    sup = work_ps.tile([128, N], F32, tag="ps")
                    nc.tensor.matmul(
                        sup[:],
                        Knat[c][:, jt * 128 : (jt + 1) * 128],
                        u[:],
                        start=True,
                        stop=True,
                    )
                    nc.vector.tensor_add(ST[jt][:], ST[jt][:], sup[:])


_CACHE: dict = {}


def _get_compiled():
    if "nc" not in _CACHE:
        nc = bacc.Bacc(
            "TRN2", target_bir_lowering=False, debug=False, num_devices=NCORES
        )
        xT = nc.dram_tensor("xT", [DIM, BL, T], F32R, kind="ExternalInput")
        wT = nc.dram_tensor("wT", [DIM, P3N], F32R, kind="ExternalInput")
        s0T = nc.dram_tensor("s0T", [BL, N, N], F32R, kind="ExternalInput")
        out = nc.dram_tensor("out", [BL, T, N], F32, kind="ExternalOutput")
        with tile.TileContext(nc) as tc, ExitStack() as ctx:
            build_body(nc, tc, ctx, xT, wT, s0T, out)
        nc.compile()
        _CACHE["nc"] = nc
    return _CACHE["nc"]


def make_in_maps(x, S0, W_k, W_v, W_q):
    x = np.ascontiguousarray(np.asarray(x, dtype=np.float32))
    S0 = np.asarray(S0, dtype=np.float32)
    wT = np.ascontiguousarray(
        np.concatenate(
            [np.asarray(W_k), np.asarray(W_v), np.asarray(W_q)], axis=0
        ).T.astype(np.float32)
    )  # [DIM, 3N]
    xT = x.transpose(2, 1, 0)  # [DIM, B, T]
    s0T = S0.transpose(0, 2, 1)  # [B, N, N] with S^T per batch
    in_maps = []
    for core in range(NCORES):
        bs = slice(core * BL, (core + 1) * BL)
        in_maps.append(
            {
                "xT": np.ascontiguousarray(xT[:, bs, :]),
                "wT": wT,
                "s0T": np.ascontiguousarray(s0T[bs]),
            }
        )
    return in_maps


def kernel(x, S0, W_k, W_v, W_q):
    nc = _get_compiled()
    in_maps = make_in_maps(x, S0, W_k, W_v, W_q)
    res = run_bass_kernel_spmd(nc, in_maps, core_ids=list(range(NCORES)))
    outs = np.concatenate([r["out"] for r in res.results], axis=0)  # [B, T, N]
    return np.ascontiguousarray(outs.transpose(1, 0, 2))  # [T, B, N]
